# revision 12
# baseline (speedup 1.0000x reference)
"""Distributed GQA attention (B=1, T=2048, D=2048, 16 Q heads / 8 KV heads,
head_dim=128, interleaved RoPE, causal) on 8 TRN2 NeuronCores.

Sharding: tensor-parallel over heads. Core c owns Q heads {2c, 2c+1} and KV
head c (exactly the GQA group). After local attention, per-(qs 512-col block)
chunks of the attention output (transposed [feat, T] layout) are AllGathered;
each core then computes a 256-column shard of the final projection with its
column slice of Wo. The host stitches the 8 column shards (transposing back).

Schedule: one long PE stream. Each projection wave ns is finely interleaved
with the S-pass of attention block qs=ns-1 (S / rowsum matmul chunks slotted
between projection groups) so the Exp activations -- the serial scalar-engine
resource -- drain at production rate without PSUM backlog. Each AV-pass runs
dense right after, ships its AllGather chunk ~2us later (softmax reciprocal +
broadcast are hoisted into the AV window), and the four output-projection
blocks run at the tail covered by the AllGather pipeline (a dummy-matmul
stretch keeps the PE p-state high while the last AllGather lands). RoPE's
even/odd pairing is baked into a host-side column permutation of Wq/Wk (no PE
permute; partner lanes come via two small SBUF->SBUF DMAs). Causally-dead
columns of diagonal S blocks are never computed (widths 512/384/256/128).
Bulk loads and gather fetches use few wide strided DMAs spread across queues.

Compute dtype: bf16 matmul inputs, f32 PSUM accumulation, f32 softmax stats.
x is marshalled host-side to transposed bf16 layout (pure relayout; all
arithmetic runs on device).
"""

import numpy as np

import concourse.bass as bass
import concourse.bass_isa as bass_isa
import concourse.mybir as mybir
from concourse import bacc, tile
from concourse.bass_utils import run_bass_kernel_spmd

F32 = mybir.dt.float32
BF16 = mybir.dt.bfloat16
NPBF16 = mybir.dt.np(BF16)

P = 128
T = 2048
D = 2048
NC = 8          # cores
HQ = 2          # q heads per core
DH = 128        # head dim
NT = T // P     # 16 k/t blocks
QS = 512        # q super-block width
NQS = T // QS   # 4
ND = D // P     # 16 feature blocks
NB = NC * HQ    # 16 gathered feature blocks
SCALE = 1.0 / float(np.sqrt(DH))


def _rope_tables():
    # Half-split layout: rows 0..63 are the even features (i), rows 64..127
    # the odd partners. out[p] = q[p]*ctab[p] + q[p^64]*stab[p].
    inv_freq = 1.0 / (10000.0 ** (np.arange(0, DH, 2, dtype=np.float64) / DH))
    ang = np.arange(T, dtype=np.float64)[None, :] * inv_freq[:, None]  # [64, T]
    cos = np.cos(ang)
    sin = np.sin(ang)
    ctab = np.empty((DH, T), np.float32)
    stab = np.empty((DH, T), np.float32)
    ctab[0:64] = cos
    ctab[64:128] = cos
    stab[0:64] = -sin   # even row: out = q_e*c - q_o*s
    stab[64:128] = sin  # odd row:  out = q_o*c + q_e*s
    return ctab.astype(NPBF16), stab.astype(NPBF16)


def _trimask():
    # [128, 128] diagonal-block mask: mask[tk, ql] = 1 if ql >= tk
    tk = np.arange(P)[:, None]
    ql = np.arange(P)[None, :]
    return (ql >= tk).astype(NPBF16)


# even features first, then their odd partners (per 128-wide head block)
_EO = np.concatenate([np.arange(0, P, 2), np.arange(1, P, 2)])


def build_nc():
    nc = bacc.Bacc(num_devices=NC)

    # x^T marshalled j-major: [P, ND*T], row p holds feature blocks j
    xt_e = nc.declare_dram_parameter("xt", [P, ND * T], BF16, isOutput=False)
    # wq is head-major: [P, HQ * ND * DH] (per-head contiguous for split DMA)
    wq_e = nc.declare_dram_parameter("wq", [P, HQ * ND * DH], BF16, isOutput=False)
    wk_e = nc.declare_dram_parameter("wk", [P, ND * DH], BF16, isOutput=False)
    wv_e = nc.declare_dram_parameter("wv", [P, ND * DH], BF16, isOutput=False)
    wo_e = nc.declare_dram_parameter("wo", [P, ND * HQ * DH], BF16, isOutput=False)
    bq_e = nc.declare_dram_parameter("bq", [HQ, P], F32, isOutput=False)
    bk_e = nc.declare_dram_parameter("bk", [1, P], F32, isOutput=False)
    bv_e = nc.declare_dram_parameter("bv", [1, P], F32, isOutput=False)
    bo_e = nc.declare_dram_parameter("bo", [HQ, P], F32, isOutput=False)
    ct_e = nc.declare_dram_parameter("costab", [DH, T], BF16, isOutput=False)
    st_e = nc.declare_dram_parameter("sintab", [DH, T], BF16, isOutput=False)
    tm_e = nc.declare_dram_parameter("trimask", [P, P], BF16, isOutput=False)
    idb_e = nc.declare_dram_parameter("identb", [P, P], BF16, isOutput=False)
    out_e = nc.declare_dram_parameter("out", [HQ * DH, T], F32, isOutput=True)

    rg = [list(range(NC))]

    with tile.TileContext(nc) as tc:
        # ---------- long-lived pools (stack order: longest-lived first) ------
        const = tc.alloc_tile_pool(name="const", bufs=1)
        identb = const.tile([P, P], BF16)
        trimask = const.tile([P, P], BF16)
        bq_t = const.tile([P, HQ], F32)
        bk_t = const.tile([P, 1], F32)
        bv_t = const.tile([P, 1], F32)
        bo_t = const.tile([P, HQ], F32)

        wpool = tc.alloc_tile_pool(name="wpool", bufs=1)
        wq_sb = wpool.tile([P, HQ * ND * DH], BF16)
        wk_sb = wpool.tile([P, ND * DH], BF16)
        wv_sb = wpool.tile([P, ND * DH], BF16)
        wo_sb = wpool.tile([P, ND * HQ * DH], BF16)

        tabp = tc.alloc_tile_pool(name="tabp", bufs=1)
        ctab = tabp.tile([DH, T], BF16)
        stab = tabp.tile([DH, T], BF16)

        rope_pool = tc.alloc_tile_pool(name="ropeo", bufs=1)
        q_r = [rope_pool.tile([P, T], BF16, name=f"qr{h}") for h in range(HQ)]
        k_r = rope_pool.tile([P, T], BF16)

        vnat_pool = tc.alloc_tile_pool(name="vnat", bufs=1)
        v_nat = [vnat_pool.tile([P, DH], BF16, name=f"vnat{n}") for n in range(NT)]

        # softmax-stat + staging pools
        accp = tc.alloc_tile_pool(name="accp", bufs=2)    # [P,QS] f32 exp sums
        rbp = tc.alloc_tile_pool(name="rbp", bufs=2)      # [P,QS] recip rows
        olocp = tc.alloc_tile_pool(name="olocp", bufs=4)
        finp = tc.alloc_tile_pool(name="finp", bufs=3)
        ptpool = tc.alloc_tile_pool(name="ptpool", bufs=32)

        # proj temp pools
        qtp = tc.alloc_tile_pool(name="qtp", bufs=3)
        vtp = tc.alloc_tile_pool(name="vtp", bufs=2)
        qswp = tc.alloc_tile_pool(name="qswp", bufs=3)
        rt1p = tc.alloc_tile_pool(name="rt1p", bufs=2)
        rt2p = tc.alloc_tile_pool(name="rt2p", bufs=2)

        dram = tc.alloc_tile_pool(name="dram", bufs=1, space="DRAM")
        agin = [dram.tile([HQ * P, QS], BF16, name=f"agin{q}")
                for q in range(NQS)]
        agout = [dram.tile([NB * P, QS], BF16, name=f"agout{q}",
                           addr_space="Shared") for q in range(NQS)]
        dramw = tc.alloc_tile_pool(name="dramw", bufs=1, space="DRAM")
        warm_in = dramw.tile([1, 256], BF16, name="warmin")
        warm_out = dramw.tile([NC, 256], BF16, name="warmout",
                              addr_space="Shared")

        # warm up the CC rings immediately (input never read downstream, so
        # garbage DRAM content is fine -- no dependency delays the barrier)
        nc.gpsimd.collective_compute(
            "AllGather", mybir.AluOpType.bypass, replica_groups=rg,
            ins=[warm_in.opt()], outs=[warm_out.opt()])

        # ---------- phase A DMA: first projection's inputs lead -------------
        # wave-0 critical inputs first, finely chopped, spread over 4 dispatch
        # queues so dispatch serialization (~0.7us per dma_start) doesn't gate
        # the first matmuls
        xT_pool = tc.alloc_tile_pool(name="xT", bufs=1)
        xT_big = xT_pool.tile([P, ND * T], BF16)
        NSW = ND * QS  # 8192 cols per ns-slab

        def xpart(ns, c0, c1, eng):
            sl = slice(ns * NSW + c0, ns * NSW + c1)
            eng.dma_start(out=xT_big[:, sl], in_=xt_e[:, sl])

        def xrhs(j, ns):
            return xT_big[:, ns * NSW + j * QS: ns * NSW + (j + 1) * QS]

        nc.sync.dma_start(out=wq_sb[:, 0:2 * DH], in_=wq_e[:, 0:2 * DH])
        xpart(0, 0, 1024, nc.sync)             # j=0,1
        nc.scalar.dma_start(out=wq_sb[:, 2 * DH:8 * DH],
                            in_=wq_e[:, 2 * DH:8 * DH])
        xpart(0, 1024, 2048, nc.scalar)        # j=2,3
        nc.gpsimd.dma_start(out=bq_t[:], in_=bq_e.rearrange("h p -> p h"))
        xpart(0, 2048, 4096, nc.sync)          # j=4..7
        nc.gpsimd.dma_start(out=ctab[:], in_=ct_e[:])
        xpart(0, 4096, 6144, nc.scalar)        # j=8..11
        nc.sync.dma_start(out=wq_sb[:, 8 * DH:ND * DH],
                          in_=wq_e[:, 8 * DH:ND * DH])
        xpart(0, 6144, 8192, nc.gpsimd)        # j=12..15
        nc.gpsimd.dma_start(out=stab[:], in_=st_e[:])
        nc.scalar.dma_start(out=wq_sb[:, ND * DH:], in_=wq_e[:, ND * DH:])
        nc.sync.dma_start(out=wk_sb[:], in_=wk_e[:])
        nc.gpsimd.dma_start(out=bk_t[:], in_=bk_e.rearrange("h p -> p h"))
        nc.gpsimd.dma_start(out=bv_t[:], in_=bv_e.rearrange("h p -> p h"))
        nc.sync.dma_start(out=wv_sb[:], in_=wv_e[:])
        nc.scalar.dma_start(out=identb[:], in_=idb_e[:])
        nc.gpsimd.dma_start(out=trimask[:], in_=tm_e[:])
        nc.gpsimd.dma_start(out=bo_t[:], in_=bo_e.rearrange("h p -> p h"))
        for ns in (1, 2, 3):
            for i in range(4):
                eng = nc.gpsimd if i % 2 else nc.sync
                xpart(ns, i * (NSW // 4), (i + 1) * (NSW // 4), eng)
        nc.sync.dma_start(out=wo_sb[:], in_=wo_e[:])

        ag_big = {}

        def fetch_ag(ns, ag_pool, split=False):
            # one wide strided DMA per 2 gathered blocks
            t = ag_pool.tile([P, NB * QS], BF16, name=f"ag{ns}")
            dst = t.rearrange("p (b q) -> p b q", b=NB)
            src = agout[ns].rearrange("(b p) q -> p b q", p=P)
            for i in range(0, NB, 2):
                eng = nc.gpsimd if (split and (i // 2) % 2) else nc.sync
                eng.dma_start(out=dst[:, i:i + 2, :],
                              in_=src[:, i:i + 2, :])
            ag_big[ns] = t

        def ag_rhs(ns, b):
            return ag_big[ns][:, b * QS:(b + 1) * QS]

        with tc.tile_pool(name="ppsum", bufs=2, space="PSUM") as ppsum, \
             tc.tile_pool(name="spsum", bufs=3, space="PSUM") as spsum, \
             tc.tile_pool(name="opsum", bufs=2, space="PSUM") as opsum:

            def rope(qt, dst, ns):
                # dst[:, sl] = qt*ctab + swap(qt)*stab  (half-split layout)
                sl = slice(ns * QS, (ns + 1) * QS)
                qsw = qswp.tile([P, QS], BF16, tag="qsw")
                nc.gpsimd.dma_start(out=qsw[0:64, :], in_=qt[64:128, :])
                nc.gpsimd.dma_start(out=qsw[64:128, :], in_=qt[0:64, :])
                t1 = rt1p.tile([P, QS], BF16, tag="t1")
                nc.vector.tensor_mul(t1[:], qt[:], ctab[:, sl])
                t2 = rt2p.tile([P, QS], BF16, tag="t2")
                nc.vector.tensor_mul(t2[:], qsw[:], stab[:, sl])
                nc.vector.tensor_add(dst[:, sl], t1[:], t2[:])

            def proj_groups(ns):
                # yields per-group emitters for the ns-th projection wave
                def qhead(h):
                    ps = ppsum.tile([P, QS], F32, tag="pp")
                    for j in range(ND):
                        nc.tensor.matmul(
                            ps[:],
                            lhsT=wq_sb[:, (h * ND + j) * DH:(h * ND + j) * DH + P],
                            rhs=xrhs(j, ns),
                            start=(j == 0), stop=(j == ND - 1))
                    qt = qtp.tile([P, QS], BF16, tag="qt", name=f"qt{ns}_{h}")
                    nc.vector.tensor_scalar_add(qt[:], ps[:], bq_t[:, h:h + 1])
                    rope(qt, q_r[h], ns)

                def kproj():
                    ps = ppsum.tile([P, QS], F32, tag="pp")
                    for j in range(ND):
                        nc.tensor.matmul(
                            ps[:], lhsT=wk_sb[:, j * DH:j * DH + P],
                            rhs=xrhs(j, ns),
                            start=(j == 0), stop=(j == ND - 1))
                    kt = qtp.tile([P, QS], BF16, tag="qt", name=f"kt{ns}")
                    nc.vector.tensor_scalar_add(kt[:], ps[:], bk_t[:, 0:1])
                    rope(kt, k_r, ns)

                def vproj():
                    ps = ppsum.tile([P, QS], F32, tag="pp")
                    for j in range(ND):
                        nc.tensor.matmul(
                            ps[:], lhsT=wv_sb[:, j * DH:j * DH + P],
                            rhs=xrhs(j, ns),
                            start=(j == 0), stop=(j == ND - 1))
                    vt = vtp.tile([P, QS], BF16, tag="vt", name=f"vt{ns}")
                    nc.scalar.activation(
                        out=vt[:], in_=ps[:],
                        func=mybir.ActivationFunctionType.Identity,
                        bias=bv_t[:, 0:1])
                    for i in range(4):
                        n = 4 * ns + i
                        vp = ppsum.tile([P, P], BF16, tag="vp",
                                        bufs=1, name=f"vp{n}")
                        nc.tensor.transpose(vp[:], vt[:, i * P:(i + 1) * P],
                                            identb[:])
                        nc.vector.tensor_copy(out=v_nat[n][:], in_=vp[:])

                yield lambda: qhead(0)
                yield lambda: qhead(1)
                yield kproj
                yield vproj

            def s_chunks(qs, pts, accs):
                # per-kb S + exp + mask emitters (paced vs the scalar Exp
                # rate); the softmax denominator accumulates on the vector
                # engine (acc += pt, f32) instead of PE rowsum matmuls
                qbase = qs * QS
                nkb = 4 * (qs + 1)

                def chunk(kb):
                    c0 = (kb - 4 * qs) * P if kb >= 4 * qs else 0
                    for h in range(HQ):
                        s_ps = spsum.tile([P, QS], F32, tag="s")
                        nc.tensor.matmul(
                            s_ps[:, c0:QS],
                            lhsT=k_r[:, kb * P:(kb + 1) * P],
                            rhs=q_r[h][:, qbase + c0:qbase + QS],
                            start=True, stop=True)
                        pt = ptpool.tile([P, QS], BF16, tag="pt",
                                         name=f"pt{qs}_{kb}_{h}")
                        nc.scalar.activation(
                            out=pt[:, c0:QS], in_=s_ps[:, c0:QS],
                            func=mybir.ActivationFunctionType.Exp, scale=SCALE)
                        if kb >= 4 * qs:
                            nc.vector.tensor_mul(pt[:, c0:c0 + P],
                                                 pt[:, c0:c0 + P], trimask[:])
                        if kb == 0:
                            acc = accp.tile([P, QS], F32, tag=f"acc{h}",
                                            name=f"acc{qs}_{h}")
                            nc.vector.tensor_copy(out=acc[:], in_=pt[:])
                            accs[h] = acc
                        else:
                            nc.vector.tensor_add(accs[h][:, c0:QS],
                                                 accs[h][:, c0:QS],
                                                 pt[:, c0:QS])
                        pts[(kb, h)] = pt

                for kb in range(nkb):
                    yield lambda kb=kb: chunk(kb)

            def interleave(groups, chunks, chunks_first=False):
                # spread chunk emitters between the projection groups
                groups = list(groups)
                chunks = list(chunks)
                ngap = len(groups)
                done = 0
                for i, g in enumerate(groups):
                    if chunks_first:
                        take = (len(chunks) * (i + 1)) // ngap
                        while done < take:
                            chunks[done]()
                            done += 1
                        g()
                    else:
                        g()
                        take = (len(chunks) * (i + 1)) // ngap
                        while done < take:
                            chunks[done]()
                            done += 1

            def av_stats(qs, accs):
                # all-reduce over partitions replicates the rowsum to all 128
                # lanes, so the reciprocal runs full-width on the DVE
                rb = {}
                for h in range(HQ):
                    rs = rbp.tile([P, QS], F32, tag="rs", name=f"rs{qs}_{h}")
                    nc.gpsimd.partition_all_reduce(
                        rs[:], accs[h][:], channels=P,
                        reduce_op=bass_isa.ReduceOp.add)
                    rbt = rbp.tile([P, QS], F32, tag="rb", name=f"rb{qs}_{h}")
                    nc.vector.reciprocal(rbt[:], rs[:])
                    rb[h] = rbt
                return rb

            def av_head(qs, h, pts, rb):
                nkb = 4 * (qs + 1)
                o_ps = opsum.tile([P, QS], F32, tag="o", name=f"ops{qs}_{h}")
                for kb in range(nkb):
                    c0 = (kb - 4 * qs) * P if kb >= 4 * qs else 0
                    nc.tensor.matmul(o_ps[:, c0:QS], lhsT=v_nat[kb][:],
                                     rhs=pts[(kb, h)][:, c0:QS],
                                     start=(kb == 0), stop=(kb == nkb - 1))
                ol = olocp.tile([P, QS], BF16, tag="ol", name=f"ol{qs}_{h}")
                nc.vector.tensor_mul(ol[:], o_ps[:], rb[h][:])
                return ol

            def av_pass(qs, pts, accs):
                rb = av_stats(qs, accs)
                for h in range(HQ):
                    ol = av_head(qs, h, pts, rb)
                    nc.gpsimd.dma_start(
                        out=agin[qs][h * P:(h + 1) * P, :], in_=ol[:])
                nc.gpsimd.collective_compute(
                    "AllGather", mybir.AluOpType.bypass,
                    replica_groups=rg,
                    ins=[agin[qs].opt()], outs=[agout[qs].opt()])

            def fin_m(ns, m):
                f_ps = ppsum.tile([P, QS], F32, tag="pp", name=f"fps{ns}_{m}")
                for b in range(NB):
                    nc.tensor.matmul(
                        f_ps[:],
                        lhsT=wo_sb[:, b * HQ * DH + m * DH:
                                   b * HQ * DH + m * DH + P],
                        rhs=ag_rhs(ns, b),
                        start=(b == 0), stop=(b == NB - 1))
                fin = finp.tile([P, QS], F32, tag="fin", name=f"fin{ns}_{m}")
                nc.vector.tensor_scalar_add(fin[:], f_ps[:], bo_t[:, m:m + 1])
                nc.sync.dma_start(
                    out=out_e[m * P:(m + 1) * P, ns * QS:(ns + 1) * QS],
                    in_=fin[:])

            def fin_block(ns):
                for m in range(HQ):
                    fin_m(ns, m)

            # ---- the one long PE stream ------------------------------------
            for g in proj_groups(0):
                g()
            avq = {}
            for qs in range(NQS - 1):
                pts = {}
                accs = {}
                interleave(proj_groups(qs + 1), s_chunks(qs, pts, accs))
                if qs < NQS - 2:
                    av_pass(qs, pts, accs)
                else:
                    avq = (pts, accs)
            # release x^T SBUF; the gathered chunks reuse it
            xT_pool.release()
            ag_pool = tc.alloc_tile_pool(name="agsb", bufs=1)
            fetch_ag(0, ag_pool)
            fetch_ag(1, ag_pool)
            av_pass(2, *avq)
            fetch_ag(2, ag_pool)
            # last attention block: S-pass paced against fin(0) filler
            pts = {}
            accs = {}
            chunks = list(s_chunks(3, pts, accs))
            for c in chunks[0:4]:
                c()
            fin_m(0, 0)
            for c in chunks[4:16]:
                c()
            fin_m(0, 1)
            av_pass(3, pts, accs)
            fetch_ag(3, ag_pool, split=True)
            # fin(1)+fin(2) are real PE work covering the last AllGather's
            # ring latency; fin(3) runs once its fetch lands
            fin_block(1)
            fin_block(2)
            fin_block(3)

        ag_pool.release()
        dramw.release()
        dram.release()
        rt2p.release()
        rt1p.release()
        qswp.release()
        vtp.release()
        qtp.release()
        ptpool.release()
        finp.release()
        olocp.release()
        rbp.release()
        accp.release()
        vnat_pool.release()
        rope_pool.release()
        tabp.release()
        wpool.release()
        const.release()

    nc.compile()
    return nc


_NC_CACHE = None


def _get_nc():
    global _NC_CACHE
    if _NC_CACHE is None:
        _NC_CACHE = build_nc()
    return _NC_CACHE


def _warr(w):
    # [D, M] -> [P, ND*M]: row p holds feature blocks j at stride M
    m = w.shape[1]
    return np.ascontiguousarray(
        w.reshape(ND, P, m).transpose(1, 0, 2).reshape(P, ND * m)).astype(NPBF16)


def _in_maps(x, Wq, bq, Wkv, bkv, Wo, bo):
    x2 = np.asarray(x, np.float32).reshape(T, D)
    # ns-major x^T: xt[p, ns*(ND*QS) + j*QS + q] = x[ns*QS+q, j*P+p]
    xt = np.ascontiguousarray(
        x2.reshape(NQS, QS, ND, P).transpose(3, 0, 2, 1).reshape(P, ND * T)
    ).astype(NPBF16)
    Wq = np.asarray(Wq, np.float32)
    Wkv = np.asarray(Wkv, np.float32)
    Wo = np.asarray(Wo, np.float32)
    bq = np.asarray(bq, np.float32)
    bkv = np.asarray(bkv, np.float32)
    bo = np.asarray(bo, np.float32)
    ctab, stab = _rope_tables()
    tm = _trimask()
    identb = np.eye(P, dtype=NPBF16)
    NKV = 8
    maps = []
    for c in range(NC):
        qc = slice(HQ * DH * c, HQ * DH * (c + 1))
        kc = slice(DH * c, DH * (c + 1))
        vc = slice(NKV * DH + DH * c, NKV * DH + DH * (c + 1))
        # head-major, even/odd-permuted Wq: [P, HQ*ND*DH]
        wq_heads = [
            _warr(Wq[:, qc][:, h * P + _EO]) for h in range(HQ)
        ]
        bq_c = bq[qc].reshape(HQ, P)[:, _EO]
        bk_c = bkv[kc].reshape(1, P)[:, _EO]
        maps.append({
            "xt": xt,
            "wq": np.ascontiguousarray(np.concatenate(wq_heads, axis=1)),
            "wk": _warr(Wkv[:, kc][:, _EO]),
            "wv": _warr(Wkv[:, vc]),
            "wo": _warr(Wo[:, qc]),
            "bq": np.ascontiguousarray(bq_c),
            "bk": np.ascontiguousarray(bk_c),
            "bv": np.ascontiguousarray(bkv[vc]).reshape(1, P),
            "bo": np.ascontiguousarray(bo[qc]).reshape(HQ, P),
            "costab": ctab, "sintab": stab, "trimask": tm,
            "identb": identb,
        })
    return maps


def _assemble(results):
    full = np.empty((T, D), np.float32)
    for c in range(NC):
        full[:, HQ * DH * c:HQ * DH * (c + 1)] = results[c]["out"].T
    return full.reshape(1, T, D)


def run(trace=False, tmpdir=None, **inputs):
    nc = _get_nc()
    maps = _in_maps(**inputs)
    res = run_bass_kernel_spmd(nc, maps, core_ids=list(range(NC)), trace=trace,
                               tmpdir=tmpdir)
    return _assemble(res.results), res


def kernel(**inputs):
    out, _ = run(trace=False, **inputs)
    return out



# revision 20
# speedup vs baseline: 1.0440x; 1.0440x over previous
"""Distributed GQA attention (B=1, T=2048, D=2048, 16 Q heads / 8 KV heads,
head_dim=128, interleaved RoPE, causal) on 8 TRN2 NeuronCores.

Sharding: tensor-parallel over heads. Core c owns Q heads {2c, 2c+1} and KV
head c (exactly the GQA group). After local attention, per-(qs 512-col block)
chunks of the attention output (transposed [feat, T] layout) are AllGathered;
each core then computes a 256-column shard of the final projection with its
column slice of Wo. The host stitches the 8 column shards (transposing back).

Schedule: one long PE stream. Each projection wave ns is finely interleaved
with the S-pass of attention block qs=ns-1 (S / rowsum matmul chunks slotted
between projection groups) so the Exp activations -- the serial scalar-engine
resource -- drain at production rate without PSUM backlog. Each AV-pass runs
dense right after, ships its AllGather chunk ~2us later (softmax reciprocal +
broadcast are hoisted into the AV window), and the four output-projection
blocks run at the tail covered by the AllGather pipeline (a dummy-matmul
stretch keeps the PE p-state high while the last AllGather lands). RoPE's
even/odd pairing is baked into a host-side column permutation of Wq/Wk (no PE
permute; partner lanes come via two small SBUF->SBUF DMAs). Causally-dead
columns of diagonal S blocks are never computed (widths 512/384/256/128).
Bulk loads and gather fetches use few wide strided DMAs spread across queues.

Compute dtype: bf16 matmul inputs, f32 PSUM accumulation, f32 softmax stats.
x is marshalled host-side to transposed bf16 layout (pure relayout; all
arithmetic runs on device).
"""

import numpy as np

import concourse.bass as bass
import concourse.mybir as mybir
from concourse import bacc, tile
from concourse.bass_utils import run_bass_kernel_spmd

F32 = mybir.dt.float32
BF16 = mybir.dt.bfloat16
NPBF16 = mybir.dt.np(BF16)

P = 128
T = 2048
D = 2048
NC = 8          # cores
HQ = 2          # q heads per core
DH = 128        # head dim
NT = T // P     # 16 k/t blocks
QS = 512        # q super-block width
NQS = T // QS   # 4
ND = D // P     # 16 feature blocks
NB = NC * HQ    # 16 gathered feature blocks
SCALE = 1.0 / float(np.sqrt(DH))


def _rope_tables():
    # Half-split layout: rows 0..63 are the even features (i), rows 64..127
    # the odd partners. out[p] = q[p]*ctab[p] + q[p^64]*stab[p].
    inv_freq = 1.0 / (10000.0 ** (np.arange(0, DH, 2, dtype=np.float64) / DH))
    ang = np.arange(T, dtype=np.float64)[None, :] * inv_freq[:, None]  # [64, T]
    cos = np.cos(ang)
    sin = np.sin(ang)
    ctab = np.empty((DH, T), np.float32)
    stab = np.empty((DH, T), np.float32)
    ctab[0:64] = cos
    ctab[64:128] = cos
    stab[0:64] = -sin   # even row: out = q_e*c - q_o*s
    stab[64:128] = sin  # odd row:  out = q_o*c + q_e*s
    return ctab.astype(NPBF16), stab.astype(NPBF16)


def _trimask():
    # [128, 128] diagonal-block mask: mask[tk, ql] = 1 if ql >= tk
    tk = np.arange(P)[:, None]
    ql = np.arange(P)[None, :]
    return (ql >= tk).astype(NPBF16)


# even features first, then their odd partners (per 128-wide head block)
_EO = np.concatenate([np.arange(0, P, 2), np.arange(1, P, 2)])


def build_nc():
    nc = bacc.Bacc(num_devices=NC)

    # x^T marshalled j-major: [P, ND*T], row p holds feature blocks j
    xt_e = nc.declare_dram_parameter("xt", [P, ND * T], BF16, isOutput=False)
    # wq is head-major: [P, HQ * ND * DH] (per-head contiguous for split DMA)
    wq_e = nc.declare_dram_parameter("wq", [P, HQ * ND * DH], BF16, isOutput=False)
    wk_e = nc.declare_dram_parameter("wk", [P, ND * DH], BF16, isOutput=False)
    wv_e = nc.declare_dram_parameter("wv", [P, ND * DH], BF16, isOutput=False)
    wo_e = nc.declare_dram_parameter("wo", [P, ND * HQ * DH], BF16, isOutput=False)
    bq_e = nc.declare_dram_parameter("bq", [HQ, P], F32, isOutput=False)
    bk_e = nc.declare_dram_parameter("bk", [1, P], F32, isOutput=False)
    bv_e = nc.declare_dram_parameter("bv", [1, P], F32, isOutput=False)
    bo_e = nc.declare_dram_parameter("bo", [HQ, P], F32, isOutput=False)
    ct_e = nc.declare_dram_parameter("costab", [DH, T], BF16, isOutput=False)
    st_e = nc.declare_dram_parameter("sintab", [DH, T], BF16, isOutput=False)
    tm_e = nc.declare_dram_parameter("trimask", [P, P], BF16, isOutput=False)
    idb_e = nc.declare_dram_parameter("identb", [P, P], BF16, isOutput=False)
    out_e = nc.declare_dram_parameter("out", [HQ * DH, T], F32, isOutput=True)

    rg = [list(range(NC))]

    with tile.TileContext(nc) as tc:
        # ---------- long-lived pools (stack order: longest-lived first) ------
        const = tc.alloc_tile_pool(name="const", bufs=1)
        identb = const.tile([P, P], BF16)
        trimask = const.tile([P, P], BF16)
        ones_col = const.tile([P, 1], BF16)
        nc.vector.memset(ones_col[:], 1.0)
        bq_t = const.tile([P, HQ], F32)
        bk_t = const.tile([P, 1], F32)
        bv_t = const.tile([P, 1], F32)
        bo_t = const.tile([P, HQ], F32)

        wpool = tc.alloc_tile_pool(name="wpool", bufs=1)
        wq_sb = wpool.tile([P, HQ * ND * DH], BF16)
        wk_sb = wpool.tile([P, ND * DH], BF16)
        wv_sb = wpool.tile([P, ND * DH], BF16)
        wo_sb = wpool.tile([P, ND * HQ * DH], BF16)

        tabp = tc.alloc_tile_pool(name="tabp", bufs=1)
        ctab = tabp.tile([DH, T], BF16)
        stab = tabp.tile([DH, T], BF16)

        rope_pool = tc.alloc_tile_pool(name="ropeo", bufs=1)
        q_r = [rope_pool.tile([P, T], BF16, name=f"qr{h}") for h in range(HQ)]
        k_r = rope_pool.tile([P, T], BF16)

        vnat_pool = tc.alloc_tile_pool(name="vnat", bufs=1)
        v_nat = [vnat_pool.tile([P, DH], BF16, name=f"vnat{n}") for n in range(NT)]

        # softmax-stat + staging pools
        rbp = tc.alloc_tile_pool(name="rbp", bufs=2)      # softmax stat tiles
        olocp = tc.alloc_tile_pool(name="olocp", bufs=4)
        finp = tc.alloc_tile_pool(name="finp", bufs=3)
        ptpool = tc.alloc_tile_pool(name="ptpool", bufs=32)

        # proj temp pools
        qtp = tc.alloc_tile_pool(name="qtp", bufs=3)
        vtp = tc.alloc_tile_pool(name="vtp", bufs=2)
        qswp = tc.alloc_tile_pool(name="qswp", bufs=3)
        rt1p = tc.alloc_tile_pool(name="rt1p", bufs=2)
        rt2p = tc.alloc_tile_pool(name="rt2p", bufs=2)

        dram = tc.alloc_tile_pool(name="dram", bufs=1, space="DRAM")
        agin = [dram.tile([HQ * P, QS], BF16, name=f"agin{q}")
                for q in range(NQS)]
        agout = [dram.tile([NB * P, QS], BF16, name=f"agout{q}",
                           addr_space="Shared") for q in range(NQS)]
        dramw = tc.alloc_tile_pool(name="dramw", bufs=1, space="DRAM")
        warm_in = dramw.tile([1, 256], BF16, name="warmin")
        warm_out = dramw.tile([NC, 256], BF16, name="warmout",
                              addr_space="Shared")

        # warm up the CC rings immediately (input never read downstream, so
        # garbage DRAM content is fine -- no dependency delays the barrier)
        nc.gpsimd.collective_compute(
            "AllGather", mybir.AluOpType.bypass, replica_groups=rg,
            ins=[warm_in.opt()], outs=[warm_out.opt()])

        # ---------- phase A DMA: first projection's inputs lead -------------
        # wave-0 critical inputs first, finely chopped, spread over 4 dispatch
        # queues so dispatch serialization (~0.7us per dma_start) doesn't gate
        # the first matmuls
        xT_pool = tc.alloc_tile_pool(name="xT", bufs=1)
        xT_big = xT_pool.tile([P, ND * T], BF16)
        NSW = ND * QS  # 8192 cols per ns-slab

        def xpart(ns, c0, c1, eng):
            sl = slice(ns * NSW + c0, ns * NSW + c1)
            eng.dma_start(out=xT_big[:, sl], in_=xt_e[:, sl])

        def xrhs(j, ns):
            return xT_big[:, ns * NSW + j * QS: ns * NSW + (j + 1) * QS]

        nc.sync.dma_start(out=wq_sb[:, 0:2 * DH], in_=wq_e[:, 0:2 * DH])
        xpart(0, 0, 1024, nc.sync)             # j=0,1
        nc.scalar.dma_start(out=wq_sb[:, 2 * DH:8 * DH],
                            in_=wq_e[:, 2 * DH:8 * DH])
        xpart(0, 1024, 2048, nc.scalar)        # j=2,3
        nc.gpsimd.dma_start(out=bq_t[:], in_=bq_e.rearrange("h p -> p h"))
        xpart(0, 2048, 4096, nc.sync)          # j=4..7
        nc.gpsimd.dma_start(out=ctab[:], in_=ct_e[:])
        xpart(0, 4096, 6144, nc.scalar)        # j=8..11
        nc.sync.dma_start(out=wq_sb[:, 8 * DH:ND * DH],
                          in_=wq_e[:, 8 * DH:ND * DH])
        xpart(0, 6144, 8192, nc.gpsimd)        # j=12..15
        nc.gpsimd.dma_start(out=stab[:], in_=st_e[:])
        nc.scalar.dma_start(out=wq_sb[:, ND * DH:], in_=wq_e[:, ND * DH:])
        nc.sync.dma_start(out=wk_sb[:], in_=wk_e[:])
        nc.gpsimd.dma_start(out=bk_t[:], in_=bk_e.rearrange("h p -> p h"))
        nc.gpsimd.dma_start(out=bv_t[:], in_=bv_e.rearrange("h p -> p h"))
        nc.sync.dma_start(out=wv_sb[:], in_=wv_e[:])
        nc.scalar.dma_start(out=identb[:], in_=idb_e[:])
        nc.gpsimd.dma_start(out=trimask[:], in_=tm_e[:])
        nc.gpsimd.dma_start(out=bo_t[:], in_=bo_e.rearrange("h p -> p h"))
        for ns in (1, 2, 3):
            for i in range(4):
                eng = nc.gpsimd if i % 2 else nc.sync
                xpart(ns, i * (NSW // 4), (i + 1) * (NSW // 4), eng)
        nc.sync.dma_start(out=wo_sb[:], in_=wo_e[:])

        ag_big = {}

        def fetch_ag(ns, ag_pool, split=False):
            # one wide strided DMA per 2 gathered blocks
            t = ag_pool.tile([P, NB * QS], BF16, name=f"ag{ns}")
            dst = t.rearrange("p (b q) -> p b q", b=NB)
            src = agout[ns].rearrange("(b p) q -> p b q", p=P)
            for i in range(0, NB, 2):
                eng = nc.gpsimd if (split and (i // 2) % 2) else nc.sync
                eng.dma_start(out=dst[:, i:i + 2, :],
                              in_=src[:, i:i + 2, :])
            ag_big[ns] = t

        def ag_rhs(ns, b):
            return ag_big[ns][:, b * QS:(b + 1) * QS]

        with tc.tile_pool(name="ppsum", bufs=2, space="PSUM") as ppsum, \
             tc.tile_pool(name="spsum", bufs=2, space="PSUM") as spsum, \
             tc.tile_pool(name="opsum", bufs=2, space="PSUM") as opsum, \
             tc.tile_pool(name="rspsum", bufs=1, space="PSUM") as rspsum:

            def rope(qt, dst, ns):
                # dst[:, sl] = qt*ctab + swap(qt)*stab  (half-split layout)
                sl = slice(ns * QS, (ns + 1) * QS)
                qsw = qswp.tile([P, QS], BF16, tag="qsw")
                nc.gpsimd.dma_start(out=qsw[0:64, :], in_=qt[64:128, :])
                nc.gpsimd.dma_start(out=qsw[64:128, :], in_=qt[0:64, :])
                t1 = rt1p.tile([P, QS], BF16, tag="t1")
                nc.vector.tensor_mul(t1[:], qt[:], ctab[:, sl])
                t2 = rt2p.tile([P, QS], BF16, tag="t2")
                nc.vector.tensor_mul(t2[:], qsw[:], stab[:, sl])
                nc.vector.tensor_add(dst[:, sl], t1[:], t2[:])

            def proj_groups(ns):
                # yields per-group emitters for the ns-th projection wave
                def qhead(h):
                    ps = ppsum.tile([P, QS], F32, tag="pp")
                    for j in range(ND):
                        nc.tensor.matmul(
                            ps[:],
                            lhsT=wq_sb[:, (h * ND + j) * DH:(h * ND + j) * DH + P],
                            rhs=xrhs(j, ns),
                            start=(j == 0), stop=(j == ND - 1))
                    qt = qtp.tile([P, QS], BF16, tag="qt", name=f"qt{ns}_{h}")
                    nc.vector.tensor_scalar_add(qt[:], ps[:], bq_t[:, h:h + 1])
                    rope(qt, q_r[h], ns)

                def kproj():
                    ps = ppsum.tile([P, QS], F32, tag="pp")
                    for j in range(ND):
                        nc.tensor.matmul(
                            ps[:], lhsT=wk_sb[:, j * DH:j * DH + P],
                            rhs=xrhs(j, ns),
                            start=(j == 0), stop=(j == ND - 1))
                    kt = qtp.tile([P, QS], BF16, tag="qt", name=f"kt{ns}")
                    nc.vector.tensor_scalar_add(kt[:], ps[:], bk_t[:, 0:1])
                    rope(kt, k_r, ns)

                def vproj():
                    ps = ppsum.tile([P, QS], F32, tag="pp")
                    for j in range(ND):
                        nc.tensor.matmul(
                            ps[:], lhsT=wv_sb[:, j * DH:j * DH + P],
                            rhs=xrhs(j, ns),
                            start=(j == 0), stop=(j == ND - 1))
                    vt = vtp.tile([P, QS], BF16, tag="vt", name=f"vt{ns}")
                    nc.scalar.activation(
                        out=vt[:], in_=ps[:],
                        func=mybir.ActivationFunctionType.Identity,
                        bias=bv_t[:, 0:1])
                    for i in range(4):
                        n = 4 * ns + i
                        vp = ppsum.tile([P, P], BF16, tag="vp",
                                        bufs=1, name=f"vp{n}")
                        nc.tensor.transpose(vp[:], vt[:, i * P:(i + 1) * P],
                                            identb[:])
                        nc.vector.tensor_copy(out=v_nat[n][:], in_=vp[:])

                yield lambda: qhead(0)
                yield lambda: qhead(1)
                yield kproj
                yield vproj

            def s_chunks(qs, pts, r2_ps):
                # per-kb S + exp + mask emitters (paced vs the scalar Exp
                # rate); rowsum matmuls trail at a 2-chunk lag so the masked
                # pt is long since written when the PE reads it, and the last
                # rowsum lands right after the wave's last exp
                qbase = qs * QS
                nkb = 4 * (qs + 1)

                def rowsum(kb):
                    c0 = (kb - 4 * qs) * P if kb >= 4 * qs else 0
                    for h in range(HQ):
                        nc.tensor.matmul(
                            r2_ps[64 * h:64 * h + 1, c0:QS],
                            lhsT=ones_col[:], rhs=pts[(kb, h)][:, c0:QS],
                            start=(kb == 0), stop=(kb == nkb - 1),
                            skip_group_check=True)

                def chunk(kb):
                    c0 = (kb - 4 * qs) * P if kb >= 4 * qs else 0
                    for h in range(HQ):
                        s_ps = spsum.tile([P, QS], F32, tag="s")
                        nc.tensor.matmul(
                            s_ps[:, c0:QS],
                            lhsT=k_r[:, kb * P:(kb + 1) * P],
                            rhs=q_r[h][:, qbase + c0:qbase + QS],
                            start=True, stop=True)
                        pt = ptpool.tile([P, QS], BF16, tag="pt",
                                         name=f"pt{qs}_{kb}_{h}")
                        nc.scalar.activation(
                            out=pt[:, c0:QS], in_=s_ps[:, c0:QS],
                            func=mybir.ActivationFunctionType.Exp, scale=SCALE)
                        if kb >= 4 * qs:
                            nc.vector.tensor_mul(pt[:, c0:c0 + P],
                                                 pt[:, c0:c0 + P], trimask[:])
                        pts[(kb, h)] = pt
                    if kb >= 2:
                        rowsum(kb - 2)
                    if kb == nkb - 1:
                        rowsum(nkb - 2)
                        rowsum(nkb - 1)

                for kb in range(nkb):
                    yield lambda kb=kb: chunk(kb)

            def interleave(groups, chunks, chunks_first=False):
                # spread chunk emitters between the projection groups
                groups = list(groups)
                chunks = list(chunks)
                ngap = len(groups)
                done = 0
                for i, g in enumerate(groups):
                    if chunks_first:
                        take = (len(chunks) * (i + 1)) // ngap
                        while done < take:
                            chunks[done]()
                            done += 1
                        g()
                    else:
                        g()
                        take = (len(chunks) * (i + 1)) // ngap
                        while done < take:
                            chunks[done]()
                            done += 1

            def av_stats(qs, r2_ps):
                # both heads' rowsums side by side in one [2, QS] tile; a
                # single fast-approx reciprocal (~5x cheaper than the exact
                # DVE reciprocal, ~18 correct bits) covers both heads
                rb = {}
                for h in range(HQ):
                    r_sb = rbp.tile([1, QS], F32, tag=f"rs{h}",
                                    name=f"rs{qs}_{h}")
                    nc.scalar.copy(out=r_sb[:],
                                   in_=r2_ps[64 * h:64 * h + 1, :])
                    ri = rbp.tile([1, QS], F32, tag=f"ri{h}",
                                  name=f"ri{qs}_{h}")
                    nc.vector.reciprocal_approx_fast(out=ri[:], in_=r_sb[:])
                    rbt = rbp.tile([P, QS], F32, tag=f"rb{h}",
                                   name=f"rb{qs}_{h}")
                    nc.gpsimd.partition_broadcast(rbt[:], ri[0:1, :])
                    rb[h] = rbt
                return rb

            def av_head(qs, h, pts, rb):
                nkb = 4 * (qs + 1)
                o_ps = opsum.tile([P, QS], F32, tag="o", name=f"ops{qs}_{h}")
                for kb in range(nkb):
                    c0 = (kb - 4 * qs) * P if kb >= 4 * qs else 0
                    nc.tensor.matmul(o_ps[:, c0:QS], lhsT=v_nat[kb][:],
                                     rhs=pts[(kb, h)][:, c0:QS],
                                     start=(kb == 0), stop=(kb == nkb - 1))
                ol = olocp.tile([P, QS], BF16, tag="ol", name=f"ol{qs}_{h}")
                nc.vector.tensor_mul(ol[:], o_ps[:], rb[h][:])
                return ol

            def av_pass(qs, pts, r2_ps):
                rb = av_stats(qs, r2_ps)
                for h in range(HQ):
                    ol = av_head(qs, h, pts, rb)
                    nc.gpsimd.dma_start(
                        out=agin[qs][h * P:(h + 1) * P, :], in_=ol[:])
                nc.gpsimd.collective_compute(
                    "AllGather", mybir.AluOpType.bypass,
                    replica_groups=rg,
                    ins=[agin[qs].opt()], outs=[agout[qs].opt()])

            def fin_m(ns, m):
                f_ps = ppsum.tile([P, QS], F32, tag="pp", name=f"fps{ns}_{m}")
                for b in range(NB):
                    nc.tensor.matmul(
                        f_ps[:],
                        lhsT=wo_sb[:, b * HQ * DH + m * DH:
                                   b * HQ * DH + m * DH + P],
                        rhs=ag_rhs(ns, b),
                        start=(b == 0), stop=(b == NB - 1))
                fin = finp.tile([P, QS], F32, tag="fin", name=f"fin{ns}_{m}")
                nc.vector.tensor_scalar_add(fin[:], f_ps[:], bo_t[:, m:m + 1])
                nc.sync.dma_start(
                    out=out_e[m * P:(m + 1) * P, ns * QS:(ns + 1) * QS],
                    in_=fin[:])

            def fin_block(ns):
                for m in range(HQ):
                    fin_m(ns, m)

            # ---- the one long PE stream ------------------------------------
            for g in proj_groups(0):
                g()
            avq = {}
            for qs in range(NQS - 1):
                pts = {}
                r2_ps = rspsum.tile([P, QS], F32, tag="r", name=f"rps{qs}")
                interleave(proj_groups(qs + 1), s_chunks(qs, pts, r2_ps))
                if qs < NQS - 2:
                    av_pass(qs, pts, r2_ps)
                else:
                    avq = (pts, r2_ps)
            # release x^T SBUF; the gathered chunks reuse it
            xT_pool.release()
            ag_pool = tc.alloc_tile_pool(name="agsb", bufs=1)
            fetch_ag(0, ag_pool)
            fetch_ag(1, ag_pool)
            av_pass(2, *avq)
            fetch_ag(2, ag_pool)
            # last attention block: S-pass paced against fin(0) filler
            pts = {}
            r2_ps = rspsum.tile([P, QS], F32, tag="r", name="rps3")
            chunks = list(s_chunks(3, pts, r2_ps))
            for c in chunks[0:4]:
                c()
            fin_m(0, 0)
            for c in chunks[4:16]:
                c()
            fin_m(0, 1)
            av_pass(3, pts, r2_ps)
            fetch_ag(3, ag_pool, split=True)
            # fin(1)+fin(2) are real PE work covering the last AllGather's
            # ring latency; fin(3) runs once its fetch lands
            fin_block(1)
            fin_block(2)
            fin_block(3)

        ag_pool.release()
        dramw.release()
        dram.release()
        rt2p.release()
        rt1p.release()
        qswp.release()
        vtp.release()
        qtp.release()
        ptpool.release()
        finp.release()
        olocp.release()
        rbp.release()
        vnat_pool.release()
        rope_pool.release()
        tabp.release()
        wpool.release()
        const.release()

    nc.compile()
    return nc


_NC_CACHE = None


def _get_nc():
    global _NC_CACHE
    if _NC_CACHE is None:
        _NC_CACHE = build_nc()
    return _NC_CACHE


def _warr(w):
    # [D, M] -> [P, ND*M]: row p holds feature blocks j at stride M
    m = w.shape[1]
    return np.ascontiguousarray(
        w.reshape(ND, P, m).transpose(1, 0, 2).reshape(P, ND * m)).astype(NPBF16)


def _in_maps(x, Wq, bq, Wkv, bkv, Wo, bo):
    x2 = np.asarray(x, np.float32).reshape(T, D)
    # ns-major x^T: xt[p, ns*(ND*QS) + j*QS + q] = x[ns*QS+q, j*P+p]
    xt = np.ascontiguousarray(
        x2.reshape(NQS, QS, ND, P).transpose(3, 0, 2, 1).reshape(P, ND * T)
    ).astype(NPBF16)
    Wq = np.asarray(Wq, np.float32)
    Wkv = np.asarray(Wkv, np.float32)
    Wo = np.asarray(Wo, np.float32)
    bq = np.asarray(bq, np.float32)
    bkv = np.asarray(bkv, np.float32)
    bo = np.asarray(bo, np.float32)
    ctab, stab = _rope_tables()
    tm = _trimask()
    identb = np.eye(P, dtype=NPBF16)
    NKV = 8
    maps = []
    for c in range(NC):
        qc = slice(HQ * DH * c, HQ * DH * (c + 1))
        kc = slice(DH * c, DH * (c + 1))
        vc = slice(NKV * DH + DH * c, NKV * DH + DH * (c + 1))
        # head-major, even/odd-permuted Wq: [P, HQ*ND*DH]
        wq_heads = [
            _warr(Wq[:, qc][:, h * P + _EO]) for h in range(HQ)
        ]
        bq_c = bq[qc].reshape(HQ, P)[:, _EO]
        bk_c = bkv[kc].reshape(1, P)[:, _EO]
        maps.append({
            "xt": xt,
            "wq": np.ascontiguousarray(np.concatenate(wq_heads, axis=1)),
            "wk": _warr(Wkv[:, kc][:, _EO]),
            "wv": _warr(Wkv[:, vc]),
            "wo": _warr(Wo[:, qc]),
            "bq": np.ascontiguousarray(bq_c),
            "bk": np.ascontiguousarray(bk_c),
            "bv": np.ascontiguousarray(bkv[vc]).reshape(1, P),
            "bo": np.ascontiguousarray(bo[qc]).reshape(HQ, P),
            "costab": ctab, "sintab": stab, "trimask": tm,
            "identb": identb,
        })
    return maps


def _assemble(results):
    full = np.empty((T, D), np.float32)
    for c in range(NC):
        full[:, HQ * DH * c:HQ * DH * (c + 1)] = results[c]["out"].T
    return full.reshape(1, T, D)


def run(trace=False, tmpdir=None, **inputs):
    nc = _get_nc()
    maps = _in_maps(**inputs)
    res = run_bass_kernel_spmd(nc, maps, core_ids=list(range(NC)), trace=trace,
                               tmpdir=tmpdir)
    return _assemble(res.results), res


def kernel(**inputs):
    out, _ = run(trace=False, **inputs)
    return out



# revision 22
# speedup vs baseline: 1.1091x; 1.0623x over previous
"""Distributed GQA attention (B=1, T=2048, D=2048, 16 Q heads / 8 KV heads,
head_dim=128, interleaved RoPE, causal) on 8 TRN2 NeuronCores.

Sharding: tensor-parallel over heads. Core c owns Q heads {2c, 2c+1} and KV
head c (exactly the GQA group). After local attention, per-(qs 512-col block)
chunks of the attention output (transposed [feat, T] layout) are AllGathered;
each core then computes a 256-column shard of the final projection with its
column slice of Wo. The host stitches the 8 column shards (transposing back).

Schedule: one long PE stream. Each projection wave ns is finely interleaved
with the S-pass of attention block qs=ns-1 (S / rowsum matmul chunks slotted
between projection groups) so the Exp activations -- the serial scalar-engine
resource -- drain at production rate without PSUM backlog. Each AV-pass runs
dense right after, ships its AllGather chunk ~2us later (softmax reciprocal +
broadcast are hoisted into the AV window), and the four output-projection
blocks run at the tail covered by the AllGather pipeline (a dummy-matmul
stretch keeps the PE p-state high while the last AllGather lands). RoPE's
even/odd pairing is baked into a host-side column permutation of Wq/Wk (no PE
permute; partner lanes come via two small SBUF->SBUF DMAs). Causally-dead
columns of diagonal S blocks are never computed (widths 512/384/256/128).
Bulk loads and gather fetches use few wide strided DMAs spread across queues.

Compute dtype: bf16 matmul inputs, f32 PSUM accumulation, f32 softmax stats.
x is marshalled host-side to transposed bf16 layout (pure relayout; all
arithmetic runs on device).
"""

import numpy as np

import concourse.bass as bass
import concourse.mybir as mybir
from concourse import bacc, tile
from concourse.bass_utils import run_bass_kernel_spmd

F32 = mybir.dt.float32
BF16 = mybir.dt.bfloat16
NPBF16 = mybir.dt.np(BF16)

P = 128
T = 2048
D = 2048
NC = 8          # cores
HQ = 2          # q heads per core
DH = 128        # head dim
NT = T // P     # 16 k/t blocks
QS = 512        # q super-block width
NQS = T // QS   # 4
ND = D // P     # 16 feature blocks
NB = NC * HQ    # 16 gathered feature blocks
SCALE = 1.0 / float(np.sqrt(DH))


def _rope_tables():
    # Half-split layout: rows 0..63 are the even features (i), rows 64..127
    # the odd partners. out[p] = q[p]*ctab[p] + q[p^64]*stab[p].
    inv_freq = 1.0 / (10000.0 ** (np.arange(0, DH, 2, dtype=np.float64) / DH))
    ang = np.arange(T, dtype=np.float64)[None, :] * inv_freq[:, None]  # [64, T]
    cos = np.cos(ang)
    sin = np.sin(ang)
    ctab = np.empty((DH, T), np.float32)
    stab = np.empty((DH, T), np.float32)
    ctab[0:64] = cos
    ctab[64:128] = cos
    stab[0:64] = -sin   # even row: out = q_e*c - q_o*s
    stab[64:128] = sin  # odd row:  out = q_o*c + q_e*s
    return ctab.astype(NPBF16), stab.astype(NPBF16)


def _trimask():
    # [128, 128] diagonal-block mask: mask[tk, ql] = 1 if ql >= tk
    tk = np.arange(P)[:, None]
    ql = np.arange(P)[None, :]
    return (ql >= tk).astype(NPBF16)


# even features first, then their odd partners (per 128-wide head block)
_EO = np.concatenate([np.arange(0, P, 2), np.arange(1, P, 2)])


def build_nc():
    nc = bacc.Bacc(num_devices=NC)

    # x^T marshalled j-major: [P, ND*T], row p holds feature blocks j
    xt_e = nc.declare_dram_parameter("xt", [P, ND * T], BF16, isOutput=False)
    # wq is head-major: [P, HQ * ND * DH] (per-head contiguous for split DMA)
    wq_e = nc.declare_dram_parameter("wq", [P, HQ * ND * DH], BF16, isOutput=False)
    wk_e = nc.declare_dram_parameter("wk", [P, ND * DH], BF16, isOutput=False)
    wv_e = nc.declare_dram_parameter("wv", [P, ND * DH], BF16, isOutput=False)
    wo_e = nc.declare_dram_parameter("wo", [P, ND * HQ * DH], BF16, isOutput=False)
    bq_e = nc.declare_dram_parameter("bq", [HQ, P], F32, isOutput=False)
    bk_e = nc.declare_dram_parameter("bk", [1, P], F32, isOutput=False)
    bv_e = nc.declare_dram_parameter("bv", [1, P], F32, isOutput=False)
    bo_e = nc.declare_dram_parameter("bo", [HQ, P], F32, isOutput=False)
    ct_e = nc.declare_dram_parameter("costab", [DH, T], BF16, isOutput=False)
    st_e = nc.declare_dram_parameter("sintab", [DH, T], BF16, isOutput=False)
    tm_e = nc.declare_dram_parameter("trimask", [P, P], BF16, isOutput=False)
    idb_e = nc.declare_dram_parameter("identb", [P, P], BF16, isOutput=False)
    out_e = nc.declare_dram_parameter("out", [HQ * DH, T], F32, isOutput=True)

    rg = [list(range(NC))]

    with tile.TileContext(nc) as tc:
        # ---------- long-lived pools (stack order: longest-lived first) ------
        const = tc.alloc_tile_pool(name="const", bufs=1)
        identb = const.tile([P, P], BF16)
        trimask = const.tile([P, P], BF16)
        ones_col = const.tile([P, 1], BF16)
        nc.vector.memset(ones_col[:], 1.0)
        bq_t = const.tile([P, HQ], F32)
        bk_t = const.tile([P, 1], F32)
        bv_t = const.tile([P, 1], F32)
        bo_t = const.tile([P, HQ], F32)

        wpool = tc.alloc_tile_pool(name="wpool", bufs=1)
        wq_sb = wpool.tile([P, HQ * ND * DH], BF16)
        wk_sb = wpool.tile([P, ND * DH], BF16)
        wv_sb = wpool.tile([P, ND * DH], BF16)
        wo_sb = wpool.tile([P, ND * HQ * DH], BF16)

        tabp = tc.alloc_tile_pool(name="tabp", bufs=1)
        ctab = tabp.tile([DH, T], BF16)
        stab = tabp.tile([DH, T], BF16)

        rope_pool = tc.alloc_tile_pool(name="ropeo", bufs=1)
        q_r = [rope_pool.tile([P, T], BF16, name=f"qr{h}") for h in range(HQ)]
        k_r = rope_pool.tile([P, T], BF16)

        vnat_pool = tc.alloc_tile_pool(name="vnat", bufs=1)
        v_nat = [vnat_pool.tile([P, DH], BF16, name=f"vnat{n}") for n in range(NT)]

        # softmax-stat + staging pools
        rbp = tc.alloc_tile_pool(name="rbp", bufs=2)      # softmax stat tiles
        olocp = tc.alloc_tile_pool(name="olocp", bufs=4)
        finp = tc.alloc_tile_pool(name="finp", bufs=3)
        ptpool = tc.alloc_tile_pool(name="ptpool", bufs=32)

        # proj temp pools
        qtp = tc.alloc_tile_pool(name="qtp", bufs=3)
        vtp = tc.alloc_tile_pool(name="vtp", bufs=2)
        qswp = tc.alloc_tile_pool(name="qswp", bufs=3)
        rt1p = tc.alloc_tile_pool(name="rt1p", bufs=2)
        rt2p = tc.alloc_tile_pool(name="rt2p", bufs=2)

        dram = tc.alloc_tile_pool(name="dram", bufs=1, space="DRAM")
        agin = [dram.tile([HQ * P, QS], BF16, name=f"agin{q}")
                for q in range(NQS)]
        agout = [dram.tile([NB * P, QS], BF16, name=f"agout{q}",
                           addr_space="Shared") for q in range(NQS)]
        dramw = tc.alloc_tile_pool(name="dramw", bufs=1, space="DRAM")
        warm_in = dramw.tile([1, 256], BF16, name="warmin")
        warm_out = dramw.tile([NC, 256], BF16, name="warmout",
                              addr_space="Shared")

        # warm up the CC rings immediately (input never read downstream, so
        # garbage DRAM content is fine -- no dependency delays the barrier)
        nc.gpsimd.collective_compute(
            "AllGather", mybir.AluOpType.bypass, replica_groups=rg,
            ins=[warm_in.opt()], outs=[warm_out.opt()])

        # ---------- phase A DMA: first projection's inputs lead -------------
        # wave-0 critical inputs first, finely chopped, spread over 4 dispatch
        # queues so dispatch serialization (~0.7us per dma_start) doesn't gate
        # the first matmuls
        xT_pool = tc.alloc_tile_pool(name="xT", bufs=1)
        xT_big = xT_pool.tile([P, ND * T], BF16)
        NSW = ND * QS  # 8192 cols per ns-slab

        def xpart(ns, c0, c1, eng):
            sl = slice(ns * NSW + c0, ns * NSW + c1)
            eng.dma_start(out=xT_big[:, sl], in_=xt_e[:, sl])

        def xrhs(j, ns):
            return xT_big[:, ns * NSW + j * QS: ns * NSW + (j + 1) * QS]

        # bulk traffic rides the sync + scalar hardware DGE rings only; the
        # gpsimd SWDGE rings stay clear for the latency-critical small DMAs
        # (rope qsw swaps, agin writes) -- bulk there stalls the rope chain
        nc.sync.dma_start(out=wq_sb[:, 0:2 * DH], in_=wq_e[:, 0:2 * DH])
        xpart(0, 0, 1024, nc.sync)             # j=0,1
        nc.scalar.dma_start(out=wq_sb[:, 2 * DH:8 * DH],
                            in_=wq_e[:, 2 * DH:8 * DH])
        xpart(0, 1024, 2048, nc.scalar)        # j=2,3
        nc.gpsimd.dma_start(out=bq_t[:], in_=bq_e.rearrange("h p -> p h"))
        xpart(0, 2048, 4096, nc.sync)          # j=4..7
        nc.gpsimd.dma_start(out=ctab[:], in_=ct_e[:])
        xpart(0, 4096, 6144, nc.scalar)        # j=8..11
        nc.sync.dma_start(out=wq_sb[:, 8 * DH:ND * DH],
                          in_=wq_e[:, 8 * DH:ND * DH])
        xpart(0, 6144, 8192, nc.sync)          # j=12..15
        nc.gpsimd.dma_start(out=stab[:], in_=st_e[:])
        nc.scalar.dma_start(out=wq_sb[:, ND * DH:], in_=wq_e[:, ND * DH:])
        nc.sync.dma_start(out=wk_sb[:], in_=wk_e[:])
        nc.gpsimd.dma_start(out=bk_t[:], in_=bk_e.rearrange("h p -> p h"))
        nc.gpsimd.dma_start(out=bv_t[:], in_=bv_e.rearrange("h p -> p h"))
        nc.sync.dma_start(out=wv_sb[:], in_=wv_e[:])
        nc.scalar.dma_start(out=identb[:], in_=idb_e[:])
        nc.gpsimd.dma_start(out=trimask[:], in_=tm_e[:])
        nc.gpsimd.dma_start(out=bo_t[:], in_=bo_e.rearrange("h p -> p h"))
        for ns in (1, 2, 3):
            for i in range(4):
                eng = nc.scalar if i % 2 else nc.sync
                xpart(ns, i * (NSW // 4), (i + 1) * (NSW // 4), eng)
        nc.sync.dma_start(out=wo_sb[:], in_=wo_e[:])

        ag_big = {}

        def fetch_ag(ns, ag_pool, split=False):
            # one wide strided DMA per 2 gathered blocks
            t = ag_pool.tile([P, NB * QS], BF16, name=f"ag{ns}")
            dst = t.rearrange("p (b q) -> p b q", b=NB)
            src = agout[ns].rearrange("(b p) q -> p b q", p=P)
            for i in range(0, NB, 2):
                eng = nc.gpsimd if (split and (i // 2) % 2) else nc.sync
                eng.dma_start(out=dst[:, i:i + 2, :],
                              in_=src[:, i:i + 2, :])
            ag_big[ns] = t

        def ag_rhs(ns, b):
            return ag_big[ns][:, b * QS:(b + 1) * QS]

        with tc.tile_pool(name="ppsum", bufs=2, space="PSUM") as ppsum, \
             tc.tile_pool(name="spsum", bufs=2, space="PSUM") as spsum, \
             tc.tile_pool(name="opsum", bufs=2, space="PSUM") as opsum, \
             tc.tile_pool(name="rspsum", bufs=1, space="PSUM") as rspsum:

            def rope(qt, dst, ns):
                # dst[:, sl] = qt*ctab + swap(qt)*stab  (half-split layout)
                sl = slice(ns * QS, (ns + 1) * QS)
                qsw = qswp.tile([P, QS], BF16, tag="qsw")
                nc.gpsimd.dma_start(out=qsw[0:64, :], in_=qt[64:128, :])
                nc.gpsimd.dma_start(out=qsw[64:128, :], in_=qt[0:64, :])
                t1 = rt1p.tile([P, QS], BF16, tag="t1")
                nc.vector.tensor_mul(t1[:], qt[:], ctab[:, sl])
                t2 = rt2p.tile([P, QS], BF16, tag="t2")
                nc.vector.tensor_mul(t2[:], qsw[:], stab[:, sl])
                nc.vector.tensor_add(dst[:, sl], t1[:], t2[:])

            def proj_groups(ns):
                # yields per-group emitters for the ns-th projection wave
                def qhead(h):
                    ps = ppsum.tile([P, QS], F32, tag="pp")
                    for j in range(ND):
                        nc.tensor.matmul(
                            ps[:],
                            lhsT=wq_sb[:, (h * ND + j) * DH:(h * ND + j) * DH + P],
                            rhs=xrhs(j, ns),
                            start=(j == 0), stop=(j == ND - 1))
                    qt = qtp.tile([P, QS], BF16, tag="qt", name=f"qt{ns}_{h}")
                    nc.vector.tensor_scalar_add(qt[:], ps[:], bq_t[:, h:h + 1])
                    rope(qt, q_r[h], ns)

                def kproj():
                    ps = ppsum.tile([P, QS], F32, tag="pp")
                    for j in range(ND):
                        nc.tensor.matmul(
                            ps[:], lhsT=wk_sb[:, j * DH:j * DH + P],
                            rhs=xrhs(j, ns),
                            start=(j == 0), stop=(j == ND - 1))
                    kt = qtp.tile([P, QS], BF16, tag="qt", name=f"kt{ns}")
                    nc.vector.tensor_scalar_add(kt[:], ps[:], bk_t[:, 0:1])
                    rope(kt, k_r, ns)

                def vproj():
                    ps = ppsum.tile([P, QS], F32, tag="pp")
                    for j in range(ND):
                        nc.tensor.matmul(
                            ps[:], lhsT=wv_sb[:, j * DH:j * DH + P],
                            rhs=xrhs(j, ns),
                            start=(j == 0), stop=(j == ND - 1))
                    vt = vtp.tile([P, QS], BF16, tag="vt", name=f"vt{ns}")
                    nc.scalar.activation(
                        out=vt[:], in_=ps[:],
                        func=mybir.ActivationFunctionType.Identity,
                        bias=bv_t[:, 0:1])
                    for i in range(4):
                        n = 4 * ns + i
                        vp = ppsum.tile([P, P], BF16, tag="vp",
                                        bufs=1, name=f"vp{n}")
                        nc.tensor.transpose(vp[:], vt[:, i * P:(i + 1) * P],
                                            identb[:])
                        nc.vector.tensor_copy(out=v_nat[n][:], in_=vp[:])

                yield lambda: qhead(0)
                yield lambda: qhead(1)
                yield kproj
                yield vproj

            def s_chunks(qs, pts, r2_ps):
                # per-kb S + exp + mask emitters (paced vs the scalar Exp
                # rate); rowsum matmuls trail at a 2-chunk lag so the masked
                # pt is long since written when the PE reads it, and the last
                # rowsum lands right after the wave's last exp
                qbase = qs * QS
                nkb = 4 * (qs + 1)

                def rowsum(kb):
                    c0 = (kb - 4 * qs) * P if kb >= 4 * qs else 0
                    for h in range(HQ):
                        nc.tensor.matmul(
                            r2_ps[64 * h:64 * h + 1, c0:QS],
                            lhsT=ones_col[:], rhs=pts[(kb, h)][:, c0:QS],
                            start=(kb == 0), stop=(kb == nkb - 1),
                            skip_group_check=True)

                def chunk(kb):
                    c0 = (kb - 4 * qs) * P if kb >= 4 * qs else 0
                    for h in range(HQ):
                        s_ps = spsum.tile([P, QS], F32, tag="s")
                        nc.tensor.matmul(
                            s_ps[:, c0:QS],
                            lhsT=k_r[:, kb * P:(kb + 1) * P],
                            rhs=q_r[h][:, qbase + c0:qbase + QS],
                            start=True, stop=True)
                        pt = ptpool.tile([P, QS], BF16, tag="pt",
                                         name=f"pt{qs}_{kb}_{h}")
                        nc.scalar.activation(
                            out=pt[:, c0:QS], in_=s_ps[:, c0:QS],
                            func=mybir.ActivationFunctionType.Exp, scale=SCALE)
                        if kb >= 4 * qs:
                            nc.vector.tensor_mul(pt[:, c0:c0 + P],
                                                 pt[:, c0:c0 + P], trimask[:])
                        pts[(kb, h)] = pt
                    if kb >= 3:
                        rowsum(kb - 3)
                    if kb == nkb - 1:
                        for t in range(max(0, nkb - 3), nkb):
                            rowsum(t)

                for kb in range(nkb):
                    yield lambda kb=kb: chunk(kb)

            def interleave(groups, chunks, chunks_first=False):
                # spread chunk emitters between the projection groups
                groups = list(groups)
                chunks = list(chunks)
                ngap = len(groups)
                done = 0
                for i, g in enumerate(groups):
                    if chunks_first:
                        take = (len(chunks) * (i + 1)) // ngap
                        while done < take:
                            chunks[done]()
                            done += 1
                        g()
                    else:
                        g()
                        take = (len(chunks) * (i + 1)) // ngap
                        while done < take:
                            chunks[done]()
                            done += 1

            def av_stats(qs, r2_ps):
                # both heads' rowsums side by side in one [2, QS] tile; a
                # single fast-approx reciprocal (~5x cheaper than the exact
                # DVE reciprocal, ~18 correct bits) covers both heads
                rb = {}
                for h in range(HQ):
                    r_sb = rbp.tile([1, QS], F32, tag=f"rs{h}",
                                    name=f"rs{qs}_{h}")
                    nc.scalar.copy(out=r_sb[:],
                                   in_=r2_ps[64 * h:64 * h + 1, :])
                    ri = rbp.tile([1, QS], F32, tag=f"ri{h}",
                                  name=f"ri{qs}_{h}")
                    nc.vector.reciprocal_approx_fast(out=ri[:], in_=r_sb[:])
                    rbt = rbp.tile([P, QS], F32, tag=f"rb{h}",
                                   name=f"rb{qs}_{h}")
                    nc.gpsimd.partition_broadcast(rbt[:], ri[0:1, :])
                    rb[h] = rbt
                return rb

            def av_head(qs, h, pts, rb):
                nkb = 4 * (qs + 1)
                o_ps = opsum.tile([P, QS], F32, tag="o", name=f"ops{qs}_{h}")
                for kb in range(nkb):
                    c0 = (kb - 4 * qs) * P if kb >= 4 * qs else 0
                    nc.tensor.matmul(o_ps[:, c0:QS], lhsT=v_nat[kb][:],
                                     rhs=pts[(kb, h)][:, c0:QS],
                                     start=(kb == 0), stop=(kb == nkb - 1))
                ol = olocp.tile([P, QS], BF16, tag="ol", name=f"ol{qs}_{h}")
                nc.vector.tensor_mul(ol[:], o_ps[:], rb[h][:])
                return ol

            def av_pass(qs, pts, r2_ps):
                rb = av_stats(qs, r2_ps)
                for h in range(HQ):
                    ol = av_head(qs, h, pts, rb)
                    nc.gpsimd.dma_start(
                        out=agin[qs][h * P:(h + 1) * P, :], in_=ol[:])
                nc.gpsimd.collective_compute(
                    "AllGather", mybir.AluOpType.bypass,
                    replica_groups=rg,
                    ins=[agin[qs].opt()], outs=[agout[qs].opt()])

            def fin_m(ns, m):
                f_ps = ppsum.tile([P, QS], F32, tag="pp", name=f"fps{ns}_{m}")
                for b in range(NB):
                    nc.tensor.matmul(
                        f_ps[:],
                        lhsT=wo_sb[:, b * HQ * DH + m * DH:
                                   b * HQ * DH + m * DH + P],
                        rhs=ag_rhs(ns, b),
                        start=(b == 0), stop=(b == NB - 1))
                fin = finp.tile([P, QS], F32, tag="fin", name=f"fin{ns}_{m}")
                nc.vector.tensor_scalar_add(fin[:], f_ps[:], bo_t[:, m:m + 1])
                nc.sync.dma_start(
                    out=out_e[m * P:(m + 1) * P, ns * QS:(ns + 1) * QS],
                    in_=fin[:])

            def fin_block(ns):
                for m in range(HQ):
                    fin_m(ns, m)

            # ---- the one long PE stream ------------------------------------
            for g in proj_groups(0):
                g()
            avq = {}
            for qs in range(NQS - 1):
                pts = {}
                r2_ps = rspsum.tile([P, QS], F32, tag="r", name=f"rps{qs}")
                interleave(proj_groups(qs + 1), s_chunks(qs, pts, r2_ps))
                if qs < NQS - 2:
                    av_pass(qs, pts, r2_ps)
                else:
                    avq = (pts, r2_ps)
            # release x^T SBUF; the gathered chunks reuse it
            xT_pool.release()
            ag_pool = tc.alloc_tile_pool(name="agsb", bufs=1)
            fetch_ag(0, ag_pool)
            fetch_ag(1, ag_pool)
            av_pass(2, *avq)
            fetch_ag(2, ag_pool)
            # last attention block: S-pass paced against fin(0) filler
            pts = {}
            r2_ps = rspsum.tile([P, QS], F32, tag="r", name="rps3")
            chunks = list(s_chunks(3, pts, r2_ps))
            for c in chunks[0:4]:
                c()
            fin_m(0, 0)
            for c in chunks[4:16]:
                c()
            fin_m(0, 1)
            av_pass(3, pts, r2_ps)
            fetch_ag(3, ag_pool, split=True)
            # fin(1)+fin(2) are real PE work covering the last AllGather's
            # ring latency; fin(3) runs once its fetch lands
            fin_block(1)
            fin_block(2)
            fin_block(3)

        ag_pool.release()
        dramw.release()
        dram.release()
        rt2p.release()
        rt1p.release()
        qswp.release()
        vtp.release()
        qtp.release()
        ptpool.release()
        finp.release()
        olocp.release()
        rbp.release()
        vnat_pool.release()
        rope_pool.release()
        tabp.release()
        wpool.release()
        const.release()

    nc.compile()
    return nc


_NC_CACHE = None


def _get_nc():
    global _NC_CACHE
    if _NC_CACHE is None:
        _NC_CACHE = build_nc()
    return _NC_CACHE


def _warr(w):
    # [D, M] -> [P, ND*M]: row p holds feature blocks j at stride M
    m = w.shape[1]
    return np.ascontiguousarray(
        w.reshape(ND, P, m).transpose(1, 0, 2).reshape(P, ND * m)).astype(NPBF16)


def _in_maps(x, Wq, bq, Wkv, bkv, Wo, bo):
    x2 = np.asarray(x, np.float32).reshape(T, D)
    # ns-major x^T: xt[p, ns*(ND*QS) + j*QS + q] = x[ns*QS+q, j*P+p]
    xt = np.ascontiguousarray(
        x2.reshape(NQS, QS, ND, P).transpose(3, 0, 2, 1).reshape(P, ND * T)
    ).astype(NPBF16)
    Wq = np.asarray(Wq, np.float32)
    Wkv = np.asarray(Wkv, np.float32)
    Wo = np.asarray(Wo, np.float32)
    bq = np.asarray(bq, np.float32)
    bkv = np.asarray(bkv, np.float32)
    bo = np.asarray(bo, np.float32)
    ctab, stab = _rope_tables()
    tm = _trimask()
    identb = np.eye(P, dtype=NPBF16)
    NKV = 8
    maps = []
    for c in range(NC):
        qc = slice(HQ * DH * c, HQ * DH * (c + 1))
        kc = slice(DH * c, DH * (c + 1))
        vc = slice(NKV * DH + DH * c, NKV * DH + DH * (c + 1))
        # head-major, even/odd-permuted Wq: [P, HQ*ND*DH]
        wq_heads = [
            _warr(Wq[:, qc][:, h * P + _EO]) for h in range(HQ)
        ]
        bq_c = bq[qc].reshape(HQ, P)[:, _EO]
        bk_c = bkv[kc].reshape(1, P)[:, _EO]
        maps.append({
            "xt": xt,
            "wq": np.ascontiguousarray(np.concatenate(wq_heads, axis=1)),
            "wk": _warr(Wkv[:, kc][:, _EO]),
            "wv": _warr(Wkv[:, vc]),
            "wo": _warr(Wo[:, qc]),
            "bq": np.ascontiguousarray(bq_c),
            "bk": np.ascontiguousarray(bk_c),
            "bv": np.ascontiguousarray(bkv[vc]).reshape(1, P),
            "bo": np.ascontiguousarray(bo[qc]).reshape(HQ, P),
            "costab": ctab, "sintab": stab, "trimask": tm,
            "identb": identb,
        })
    return maps


def _assemble(results):
    full = np.empty((T, D), np.float32)
    for c in range(NC):
        full[:, HQ * DH * c:HQ * DH * (c + 1)] = results[c]["out"].T
    return full.reshape(1, T, D)


def run(trace=False, tmpdir=None, **inputs):
    nc = _get_nc()
    maps = _in_maps(**inputs)
    res = run_bass_kernel_spmd(nc, maps, core_ids=list(range(NC)), trace=trace,
                               tmpdir=tmpdir)
    return _assemble(res.results), res


def kernel(**inputs):
    out, _ = run(trace=False, **inputs)
    return out

